# revision 1
# baseline (speedup 1.0000x reference)
"""Additive attention TRN2 kernel: sine-separable tanh approximation (R=6).

tanh(x) ~= sum_r b_r sin(w_r x); sin(w(a+b)) = sin(wa)cos(wb)+cos(wa)sin(wb)
collapses the B*Q*K*H tanh tensor into 2R rank-128 matmuls.

Pipeline (h on partitions everywhere):
  PE:  warm-up dummies (HAM 8/8) -> scaled projections u_r = (w_r/2pi)W x
       (r0 lands in the idle s banks, r1..5 in u waves) -> rank matmuls
       r-major as each r's tiles complete -> final attention matmuls
  DVE: scale W^T copies by w_r/2pi -> FRAC_CENTER_ANT custom op (exact
       fp32 round-to-nearest range reduction, phase 0 / 0.25 cycles)
  ACT: r0 sin/cos directly from PSUM (w_0=0.31 never leaves the Sin
       table range; cos via +pi/2 bias) -> per-wave Sin over reduced
       args -> b_r*w_v folding via Copy-with-scale -> exp with fused
       accum_out sums (softmax over q; max-subtraction skipped, |s|<12)
"""

import numpy as np
from contextlib import ExitStack

import concourse.bass as bass
import concourse.mybir as mybir
import concourse.tile as tile
from concourse import bacc
from concourse.bass_utils import run_bass_kernel_spmd

B, Q, K, D = 8, 256, 256, 128
NCORES = 8
R = 6
F16 = mybir.dt.float16
F32 = mybir.dt.float32
AF = mybir.ActivationFunctionType
PI = float(np.pi)
MAGIC = 1.5 * 2.0 ** 23

_NC = None
_FRAC_OP = None


def _register_frac_op():
    """FRAC_CENTER_ANT: out = v - ((v + C1) - C1), v = Src0 + C0.
    C1 = 1.5*2^23 makes the inner add/sub an exact fp32 round-to-nearest,
    so out = centered fractional part of (u + phase), in [-0.5, 0.5]."""
    global _FRAC_OP
    if _FRAC_OP is not None:
        return _FRAC_OP
    import concourse.dve_ops as D
    from concourse.dve_spec import Spec, Src0, C0, C1, lower
    from concourse.dve_uop import DveOpSpec

    name = "FRAC_CENTER_ANT"
    for op in D.OPS:
        if op.name == name:
            _FRAC_OP = op
            return op

    def ref(in0, in1, s0, s1, imm2):
        f32 = np.float32
        v = (in0.astype(f32) + f32(s0)).astype(f32)
        a = (v + f32(s1)).astype(f32)
        r = (a - f32(s1)).astype(f32)
        return (v - r).astype(f32)

    v = Src0 + C0
    spec = Spec(body=v - ((v + C1) - C1), reference=ref)
    row = max(D._SUB_OPCODE_FOR_NAME.values()) + 1
    shas = {}
    for ver in ("v3", "v4"):
        try:
            r_ = DveOpSpec(name=name, opcode=row, uops=lower(spec, ver=ver),
                           rd1_en=False)
            shas[ver] = r_.sha(ver)
        except Exception:
            pass
    op = D.DveOp(name, spec, subdim=False, uops_sha=shas)
    D.OPS.append(op)
    D.CUSTOM_DVE_SPECS[name] = spec
    D._SUB_OPCODE_FOR_NAME[name] = row
    _FRAC_OP = op
    return op


# tanh(x) ~= sum_r B_COEF[r] * sin(OMEGAS[r] * x): weighted-minimax fit on
# x in [-8.5, 8.5] with N(0, 0.8165^2) density weighting (x = qh + kh);
# max abs err ~2e-3 where the data lives, bounded (sum|b| = 1.7) everywhere.
B_COEF = [1.225494035224848, 0.30446256083002976, 0.10701407413708372,
          0.038794977431962537, 0.0135932114637894, 0.004989526037730243]
OMEGAS = [0.31493161943846565, 0.9521459851525309, 1.607671337241628,
          2.2870019672036266, 3.012586730670448, 4.0557629789970715]


def _fit_params():
    return np.asarray(B_COEF, np.float64), np.asarray(OMEGAS, np.float64)


def _build_nc(omegas):
    frac_op = _register_frac_op()
    nc = bacc.Bacc("TRN2", target_bir_lowering=False)

    qT_d = nc.dram_tensor("qT", [D, Q], F16, kind="ExternalInput")
    kT_d = nc.dram_tensor("kT", [D, K], F16, kind="ExternalInput")
    WT_d = nc.dram_tensor("WT", [D, 2, D], F16, kind="ExternalInput")
    wvb_d = nc.dram_tensor("wvb", [D, R], F32, kind="ExternalInput")
    vals_d = nc.dram_tensor("vals", [K, D], F32, kind="ExternalInput")
    out_d = nc.dram_tensor("out", [Q, D], F16, kind="ExternalOutput")

    with tile.TileContext(nc) as tc, ExitStack() as ctx:
        consts = ctx.enter_context(tc.tile_pool(name="consts", bufs=1))
        u_pool = ctx.enter_context(tc.tile_pool(name="u_ps", bufs=2, space="PSUM"))
        s_pool = ctx.enter_context(tc.tile_pool(name="s_ps", bufs=2, space="PSUM"))
        o_pool = ctx.enter_context(tc.tile_pool(name="o_ps", bufs=2, space="PSUM"))

        o_tiles = [o_pool.tile([D, D], F32, tag="o", name=f"o_ps{c}")
                   for c in range(2)]
        s_tiles = [s_pool.tile([D, Q], F32, tag="s", name=f"s_ps{c}")
                   for c in range(2)]

        # ---- loads, balanced across the three DMA dispatch queues so the
        # W chunks and the qT/kT halves all transfer in parallel (queue
        # bandwidth, not dispatch, is the limiter at kernel start)
        WT_sb = consts.tile([D, 2, D], F16, tag="WT")
        qT_sb = consts.tile([D, Q], F16, tag="qT")
        kT_sb = consts.tile([D, K], F16, tag="kT")
        wvb_sb = consts.tile([D, R], F32, tag="wvb")
        vals_sb = consts.tile([D, 2, D], F32, tag="vals")
        nc.scalar.dma_start(WT_sb[:, 0, :], WT_d[:, 0, :])
        nc.sync.dma_start(qT_sb[:, 0:D], qT_d[:, 0:D])
        nc.gpsimd.dma_start(qT_sb[:, D:Q], qT_d[:, D:Q])
        nc.scalar.dma_start(WT_sb[:, 1, :], WT_d[:, 1, :])
        nc.sync.dma_start(kT_sb[:, 0:D], kT_d[:, 0:D])
        nc.gpsimd.dma_start(kT_sb[:, D:Q], kT_d[:, D:Q])
        nc.scalar.dma_start(wvb_sb[:], wvb_d[:])
        nc.sync.dma_start(vals_sb[:], vals_d.rearrange("(c p) v -> p c v", p=D))

        # ---- scale W on device: WS[:, side, r, :] = WT[:, side, :]*(w_r/2pi)
        WS_sb = consts.tile([D, 2, R, D], F16, tag="WS")
        for side in range(2):
            for r in range(R):
                nc.vector.tensor_scalar_mul(
                    WS_sb[:, side, r, :], WT_sb[:, side, :],
                    float(omegas[r] / (2 * np.pi)))

        # f/sc layout: [side, r, phase, x], phase 0=sin, 1=cos
        f_sb = consts.tile([D, 2, R, 2, Q], F32, tag="f")
        sc_sb = consts.tile([D, 2, R, 2, Q], F16, tag="sc")
        e_sb = consts.tile([D, 2, Q], F16, tag="e")
        sums_sb = consts.tile([D, 4], F32, tag="sums")
        vscaled_sb = consts.tile([D, 2, D], F16, tag="vscaled")
        out_sb = consts.tile([D, 2, D], F16, tag="outsb")

        xT = {0: qT_sb, 1: kT_sb}
        pi2_sb = consts.tile([D, 1], F32, tag="pi2")
        nc.vector.memset(pi2_sb[:], PI / 2)

        # ---- r0 projections into the (idle until rank-matmul time) s banks.
        # w_0 = 0.31 never leaves the Sin table's valid range, so r0 skips
        # range reduction entirely: its sin/cos read the PSUM tile directly.
        for side in range(2):
            nc.tensor.matmul(s_tiles[side][:], WS_sb[:, side, 0, :],
                             xT[side][:], start=True, stop=True)

        # ---- waves: r1-2 (1 PSUM bank) then r3-5 (2 banks), per side.
        # Projections of a wave-pair are emitted back-to-back before their
        # FRACs so the PE runs them without waiting on DVE progress; the
        # wvb folds are interleaved into the FRAC chain at points where
        # their sin inputs are already available.
        WAVES = [(0, 1, 3), (1, 1, 3), (0, 3, 6), (1, 3, 5), (1, 5, 6)]
        u_tiles = []
        def emit_projs(wv_i):
            side, lo, hi = WAVES[wv_i]
            u_ps = u_pool.tile([D, 3, Q], F32, tag="u", name=f"u{wv_i}")
            u_tiles.append(u_ps)
            for i in range(hi - lo):
                nc.tensor.matmul(u_ps[:, i, :], WS_sb[:, side, lo + i, :],
                                 xT[side][:], start=True, stop=True)
        def emit_fracs(wv_i):
            side, lo, hi = WAVES[wv_i]
            u_ps = u_tiles[wv_i]
            uflat = u_ps[:, :hi - lo, :].rearrange("p a x -> p (a x)")
            for ph, phase in enumerate((0.0, 0.25)):
                nc.vector._custom_dve(
                    frac_op,
                    out=f_sb[:, side, lo:hi, ph, :],
                    in0=uflat, s0=phase, s1=MAGIC)
        def emit_mul(r):
            nc.vector.tensor_scalar_mul(
                sc_sb[:, 0, r], sc_sb[:, 0, r], wvb_sb[:, r:r + 1])

        def emit_sins(wv_i):
            side, lo, hi = WAVES[wv_i]
            for ph in range(2):
                nc.scalar.activation(sc_sb[:, side, lo:hi, ph, :],
                                     f_sb[:, side, lo:hi, ph, :], AF.Sin,
                                     scale=2 * PI)

        # r0 sin/cos directly from the s-bank projections (in-range args)
        for side in range(2):
            nc.scalar.activation(sc_sb[:, side, 0, 0, :], s_tiles[side][:],
                                 AF.Sin, scale=2 * PI)
            nc.scalar.activation(sc_sb[:, side, 0, 1, :], s_tiles[side][:],
                                 AF.Sin, scale=2 * PI, bias=pi2_sb[:])

        emit_projs(0); emit_projs(1)
        emit_fracs(0); emit_fracs(1)
        emit_sins(0); emit_sins(1)
        emit_projs(2); emit_projs(3)
        emit_mul(0); emit_mul(1); emit_mul(2)
        emit_fracs(2); emit_fracs(3)
        emit_sins(2); emit_sins(3)
        emit_projs(4)
        emit_fracs(4)
        emit_sins(4)
        emit_mul(3); emit_mul(4); emit_mul(5)

        # ---- rank matmuls r-major (each r starts as soon as its tiles and
        # fold are ready, keeping PE warm through the sin stretch)
        ndone = [0, 0]
        def rank_mm(r, ph, kc):
            nc.tensor.matmul(
                s_tiles[kc][:],
                sc_sb[:, 1, r, 1 - ph, kc * D:(kc + 1) * D],
                sc_sb[:, 0, r, ph, :],
                start=(ndone[kc] == 0), stop=(ndone[kc] == 2 * R - 1))
            ndone[kc] += 1
        for r in range(R - 1):
            for ph in range(2):
                for kc in range(2):
                    rank_mm(r, ph, kc)
        for kc in range(2):
            for ph in range(2):
                rank_mm(R - 1, ph, kc)

        # ---- softmax over q (free axis) + normalization folded into values
        for kc in range(2):
            nc.scalar.activation(e_sb[:, kc, :], s_tiles[kc][:], AF.Exp,
                                 accum_out=sums_sb[:, kc:kc + 1])
            nc.vector.reciprocal(sums_sb[:, 2 + kc:3 + kc], sums_sb[:, kc:kc + 1])
            nc.vector.tensor_scalar_mul(
                vscaled_sb[:, kc, :], vals_sb[:, kc, :],
                sums_sb[:, 2 + kc:3 + kc])

        # ---- out[q, v] = sum_k e_T[k, q] * values'[k, v]
        for qh in range(2):
            for kc in range(2):
                nc.tensor.matmul(
                    o_tiles[qh][:],
                    e_sb[:, kc, qh * D:(qh + 1) * D],
                    vscaled_sb[:, kc, :],
                    start=(kc == 0), stop=(kc == 1))
            nc.vector.tensor_copy(out_sb[:, qh, :], o_tiles[qh][:])
        nc.sync.dma_start(out_d.rearrange("(c p) v -> p c v", p=D), out_sb[:])

    nc.compile()
    return nc


def _prep_in_maps(inputs):
    q = np.asarray(inputs["queries"], dtype=np.float32)
    k = np.asarray(inputs["keys"], dtype=np.float32)
    v = np.asarray(inputs["values"], dtype=np.float32)
    Wq = np.asarray(inputs["W_q"], dtype=np.float32)
    Wk = np.asarray(inputs["W_k"], dtype=np.float32)
    wv = np.asarray(inputs["w_v"], dtype=np.float32)

    b, om = _fit_params()
    WT = np.stack([Wq.T, Wk.T], axis=1).astype(np.float16)
    wvb = (wv[:, None].astype(np.float64) * b[None, :]).astype(np.float32)

    qT = q.transpose(0, 2, 1).astype(np.float16)
    kT = k.transpose(0, 2, 1).astype(np.float16)

    in_maps = []
    for bi in range(NCORES):
        in_maps.append({
            "qT": np.ascontiguousarray(qT[bi]),
            "kT": np.ascontiguousarray(kT[bi]),
            "vals": np.ascontiguousarray(v[bi]),
            "WT": np.ascontiguousarray(WT),
            "wvb": wvb,
        })
    return in_maps


def get_nc():
    global _NC
    if _NC is None:
        _, om = _fit_params()
        _NC = _build_nc(om)
    return _NC


def run(inputs, trace=False):
    nc = get_nc()
    in_maps = _prep_in_maps(inputs)
    res = run_bass_kernel_spmd(nc, in_maps, list(range(NCORES)), trace=trace)
    out = np.stack([res.results[i]["out"] for i in range(NCORES)], axis=0)
    return np.ascontiguousarray(out.astype(np.float32)), res


def kernel(**inputs):
    out, _ = run(inputs, trace=False)
    return out

